# revision 6
# baseline (speedup 1.0000x reference)
"""Bass/Trainium2 kernel for nn_PhysicsLoss — local_scatter all-local design.

Per core (400K edges), two node-partitioned views (by src and by dst, 783
nodes per partition). Per side: expand own endpoint's voltage over sorted
runs (local_scatter + hold-scan), exchange the other endpoint's voltage via
a padded block layout + DRAM block-transpose (plain strided DMA), compute
per-edge current densely, segment-sum via run-local scan + one local_scatter
straight into node slots. AllReduce the [100224] node accumulator + KVL
partial sums; finish the loss on device. No per-element indirect DMAs.
"""
import numpy as np

P = 128
NPP = 783
ACC_ROWS = P * NPP        # 100224
N_NODES = 100000
N_EDGES = 3200000
NCORES = 8
EPC = N_EDGES // NCORES   # 400000
CW = 3392                 # padded per-partition edge columns
HW_ = CW // 2             # 1696
WB = 52                   # padded block width
BW = P * WB               # 7168
QW = BW // 4              # 1664
EPS = 1e-6
PADLOG = -40.0

_cache = {}
_last_in_maps = None


def _build():
    import concourse.bass as bass
    import concourse.bacc as bacc
    import concourse.mybir as mybir
    from concourse.tile import TileContext

    f32 = mybir.dt.float32
    bf16 = mybir.dt.bfloat16
    i16 = mybir.dt.int16
    OP = mybir.AluOpType
    AF = mybir.ActivationFunctionType

    nc = bacc.Bacc("TRN2", target_bir_lowering=False, debug=False, num_devices=NCORES)

    # ---- inputs ----
    v_d = nc.dram_tensor("vloc", [P, 784], bf16, kind="ExternalInput")
    ins = {}
    for X in ("S", "D"):
        ins[X] = {
            "t2cLO": nc.dram_tensor(f"t2cLO{X}", [P, 784], i16, kind="ExternalInput"),
            "t2cHI": nc.dram_tensor(f"t2cHI{X}", [P, 784], i16, kind="ExternalInput"),
            "omff": nc.dram_tensor(f"omff{X}", [P, CW], bf16, kind="ExternalInput"),
            "e2n": nc.dram_tensor(f"e2n{X}", [P, CW], i16, kind="ExternalInput"),
            "s2b": [nc.dram_tensor(f"s2b{t}{X}", [P, CW], i16, kind="ExternalInput")
                    for t in range(4)],
            "b2xLO": nc.dram_tensor(f"b2xLO{X}", [P, BW], i16, kind="ExternalInput"),
            "b2xHI": nc.dram_tensor(f"b2xHI{X}", [P, BW], i16, kind="ExternalInput"),
            "logits": nc.dram_tensor(f"logits{X}", [P, CW], bf16, kind="ExternalInput"),
            "params": nc.dram_tensor(f"params{X}", [P, 2 * CW], bf16, kind="ExternalInput"),
        }
    out_d = nc.dram_tensor("out", [1, 1], f32, kind="ExternalOutput")

    # ---- internal DRAM ----
    ex1_d = nc.dram_tensor("ex1", [P * BW, 1], bf16)   # S-sent vsrc (B_S layout)
    ex2_d = nc.dram_tensor("ex2", [P * BW, 1], bf16)   # D-sent vdst (B_D layout)
    acc_d = nc.dram_tensor("acc_local", [ACC_ROWS + 8, 1], bf16)
    accr_d = nc.dram_tensor("acc_red", [ACC_ROWS + 8, 1], bf16)

    acc_2d = acc_d[0:ACC_ROWS, :].rearrange("(p c) o -> p (c o)", p=P)
    accr_2d = accr_d[0:ACC_ROWS, :].rearrange("(p c) o -> p (c o)", p=P)
    acc_prt = acc_d[ACC_ROWS:ACC_ROWS + 8, :].rearrange("(a b) o -> a (b o)", a=1)
    accr_prt = accr_d[ACC_ROWS:ACC_ROWS + 8, :].rearrange("(a b) o -> a (b o)", a=1)
    ex1_wr = ex1_d[:, :].rearrange("(q b) o -> q (b o)", q=P)
    ex2_wr = ex2_d[:, :].rearrange("(p b) o -> p (b o)", p=P)
    # transposed reads: recvD (on D side) reads ex1 as [p, q, r]; recvS reads ex2 as [q, p, r]
    ex1_rd = ex1_d[:, :].rearrange("(q p r) o -> p q (r o)", q=P, p=P)
    ex2_rd = ex2_d[:, :].rearrange("(p q r) o -> q p (r o)", p=P, q=P)

    with TileContext(nc) as tc:
        with (
            tc.tile_pool(name="big", bufs=1) as big,
            tc.tile_pool(name="sm", bufs=1) as sm,
            tc.tile_pool(name="ps", bufs=1, space="PSUM") as ps,
        ):
            ld = nc.sync.dma_start

            vloc = sm.tile([P, 784], bf16, tag="vloc")
            ld(out=vloc[:, :], in_=v_d[:, :])

            # shared tiles
            scr = big.tile([P, CW], f32, tag="scr")           # f32 scratch
            Lg = big.tile([P, CW], bf16, tag="Lg")            # logits landing
            Par = big.tile([P, 2 * CW], bf16, tag="Par")      # params landing
            dX = big.tile([P, CW], bf16, tag="dX")            # expand seed
            Bt = big.tile([P, BW], bf16, tag="Bt")            # block / recv tile
            curt = big.tile([P, CW], bf16, tag="curt")        # per-edge current
            rst = big.tile([P, CW], bf16, tag="rst")          # run scan
            s2bts = [big.tile([P, CW], i16, name=f"s2bt{i}", tag=f"s2bt{i}")
                     for i in range(2)]
            b2xt = big.tile([P, BW], i16, tag="b2xt")
            Bt2 = big.tile([P, BW], bf16, tag="Bt2")          # recvD buffer

            # per-side persistent tiles
            omff = {}; wtb = {}; vexp = {}; vrecv = {}; t2c = {}; node = {}
            for X in ("S", "D"):
                omff[X] = big.tile([P, CW], bf16, name=f"omff{X}", tag=f"omff{X}")
                ld(out=omff[X][:, :], in_=ins[X]["omff"][:, :])
                wtb[X] = big.tile([P, CW], bf16, name=f"w{X}", tag=f"w{X}")
                vexp[X] = big.tile([P, CW], bf16, name=f"vexp{X}", tag=f"vexp{X}")
                vrecv[X] = big.tile([P, CW], bf16, name=f"vrecv{X}", tag=f"vrecv{X}")
                t2c[X] = (sm.tile([P, 784], i16, name=f"t2cLO{X}", tag=f"t2cLO{X}"),
                          sm.tile([P, 784], i16, name=f"t2cHI{X}", tag=f"t2cHI{X}"))
                ld(out=t2c[X][0][:, :], in_=ins[X]["t2cLO"][:, :])
                ld(out=t2c[X][1][:, :], in_=ins[X]["t2cHI"][:, :])
                node[X] = sm.tile([P, 784], bf16, name=f"node{X}", tag=f"node{X}")

            # ---- expand own endpoint voltage per side ----
            for X in ("S", "D"):
                nc.gpsimd.local_scatter(
                    out_ap=dX[:, 0:HW_], data_ap=vloc[:, :], idxs_ap=t2c[X][0][:, :],
                    channels=P, num_elems=HW_, num_idxs=784)
                nc.gpsimd.local_scatter(
                    out_ap=dX[:, HW_:CW], data_ap=vloc[:, :], idxs_ap=t2c[X][1][:, :],
                    channels=P, num_elems=HW_, num_idxs=784)
                nc.vector.tensor_tensor_scan(
                    out=vexp[X][:, :], data0=omff[X][:, :], data1=dX[:, :],
                    initial=0.0, op0=OP.mult, op1=OP.add)

            # ---- KVL partials + weights ----
            prt = sm.tile([1, 8], f32, tag="prt")
            nc.vector.memset(prt[:, :], 0.0)
            red = sm.tile([P, 1], f32, tag="red")
            ones = sm.tile([P, 1], f32, tag="ones")
            nc.vector.memset(ones[:, :], 1.0)
            pssc = ps.tile([1, 1], f32, tag="pssc")

            for X in ("S", "D"):
                ld(out=Par[:, :], in_=ins[X]["params"][:, :])
                ld(out=Lg[:, :], in_=ins[X]["logits"][:, :])
                if X == "S":
                    for k in range(4):  # R, R^2, X, X^2 (padding is zero)
                        colap = Par[:, (k // 2) * CW:(k // 2 + 1) * CW]
                        if k % 2 == 0:
                            nc.vector.tensor_reduce(
                                out=red[:, :], in_=colap,
                                axis=mybir.AxisListType.X, op=OP.add)
                        else:
                            nc.scalar.activation(scr[:, :], colap, AF.Square)
                            nc.vector.tensor_reduce(
                                out=red[:, :], in_=scr[:, :],
                                axis=mybir.AxisListType.X, op=OP.add)
                        nc.tensor.matmul(pssc[:, :], lhsT=ones[:, :], rhs=red[:, :],
                                         start=True, stop=True)
                        nc.vector.tensor_copy(prt[:, k:k + 1], pssc[:, :])
                # w = sigmoid(logit) / (R + X + eps)
                nc.vector.tensor_tensor(
                    out=scr[:, :], in0=Par[:, 0:CW], in1=Par[:, CW:2 * CW],
                    op=OP.add)
                nc.vector.tensor_scalar_add(scr[:, :], scr[:, :], EPS)
                nc.vector.reciprocal(scr[:, :], scr[:, :])
                nc.scalar.activation(wtb[X][:, :], Lg[:, :], AF.Sigmoid)
                nc.vector.tensor_tensor(
                    out=wtb[X][:, :], in0=wtb[X][:, :], in1=scr[:, :], op=OP.mult)

            # ---- exchange: send own vexp into block layout, bounce via DRAM ----
            for si, (X, exwr) in enumerate((("S", ex1_wr), ("D", ex2_wr))):
                for t in range(4):
                    i = (si * 4 + t) % 2
                    ld(out=s2bts[i][:, :], in_=ins[X]["s2b"][t][:, :])
                    nc.gpsimd.local_scatter(
                        out_ap=Bt[:, t * QW:(t + 1) * QW], data_ap=vexp[X][:, :],
                        idxs_ap=s2bts[i][:, :], channels=P, num_elems=QW, num_idxs=CW)
                nc.sync.dma_start(out=exwr, in_=Bt[:, :])
                if X == "S":
                    # early transposed read of S's block tile into Bt2 (recvD)
                    Pr3 = Bt2[:, :].rearrange("p (q r) -> p q r", q=P)
                    nc.sync.dma_start(out=Pr3, in_=ex1_rd)

            # ---- receive D (from Par), then receive S (from Bt) ----
            rs2 = {}
            curD = curt
            curS = big.tile([P, CW], bf16, tag="curS")
            rstS = big.tile([P, CW], bf16, tag="rstS")
            Bt3 = Bt[:, :].rearrange("p (q r) -> p q r", q=P)
            for X, buf in (("D", Bt2[:, :]), ("S", Bt[:, :])):
                if X == "S":
                    nc.sync.dma_start(out=Bt3, in_=ex2_rd)
                ld(out=b2xt[:, :], in_=ins[X]["b2xLO"][:, :])
                nc.gpsimd.local_scatter(
                    out_ap=vrecv[X][:, 0:HW_], data_ap=buf, idxs_ap=b2xt[:, :],
                    channels=P, num_elems=HW_, num_idxs=BW)
                ld(out=b2xt[:, :], in_=ins[X]["b2xHI"][:, :])
                nc.gpsimd.local_scatter(
                    out_ap=vrecv[X][:, HW_:CW], data_ap=buf, idxs_ap=b2xt[:, :],
                    channels=P, num_elems=HW_, num_idxs=BW)
                # current + run-scan on DVE/ACT overlap the next side's gpsimd work
                ct = curD if X == "D" else curS
                rt = rst if X == "D" else rstS
                for h in (slice(0, HW_), slice(HW_, CW)):
                    nc.vector.tensor_tensor(
                        out=scr[:, h], in0=vexp[X][:, h], in1=vrecv[X][:, h],
                        op=OP.subtract)
                    nc.scalar.activation(ct[:, h], scr[:, h], AF.Abs)
                    nc.vector.tensor_tensor(
                        out=ct[:, h], in0=ct[:, h], in1=wtb[X][:, h], op=OP.mult)
                    nc.vector.tensor_tensor_scan(
                        out=rt[:, h], data0=omff[X][:, h], data1=ct[:, h],
                        initial=0.0, op0=OP.mult, op1=OP.add)
                rs2[X] = rt

            # ---- node scatters (halved; crossing run summed via two tiles) ----
            nodeH = {}
            for X in ("D", "S"):
                nodeH[X] = sm.tile([P, 784], bf16, name=f"nodeH{X}",
                                   tag=f"nodeH{X}")
                st = s2bts[0 if X == "D" else 1]
                ld(out=st[:, :], in_=ins[X]["e2n"][:, :])
                nc.gpsimd.local_scatter(
                    out_ap=node[X][:, :], data_ap=rs2[X][:, 0:HW_],
                    idxs_ap=st[:, 0:HW_],
                    channels=P, num_elems=784, num_idxs=HW_)
                nc.gpsimd.local_scatter(
                    out_ap=nodeH[X][:, :], data_ap=rs2[X][:, HW_:CW],
                    idxs_ap=st[:, HW_:CW],
                    channels=P, num_elems=784, num_idxs=HW_)

            # ---- partial = in_sum - out_sum; accumulate + reduce ----
            partial = sm.tile([P, NPP], bf16, tag="partial")
            nc.vector.tensor_tensor(
                out=partial[:, :], in0=node["D"][:, 0:NPP],
                in1=nodeH["D"][:, 0:NPP], op=OP.add)
            nc.vector.tensor_tensor(
                out=partial[:, :], in0=partial[:, :], in1=node["S"][:, 0:NPP],
                op=OP.subtract)
            nc.vector.tensor_tensor(
                out=partial[:, :], in0=partial[:, :], in1=nodeH["S"][:, 0:NPP],
                op=OP.subtract)
            nc.sync.dma_start(out=acc_2d, in_=partial[:, :])
            prtb = sm.tile([1, 8], bf16, tag="prtb")
            nc.vector.tensor_copy(prtb[:, :], prt[:, :])
            nc.sync.dma_start(out=acc_prt, in_=prtb[:, :])

            nc.gpsimd.collective_compute(
                "AllReduce", OP.add, replica_groups=[list(range(NCORES))],
                ins=[acc_d.ap().opt()], outs=[accr_d.ap().opt()])

            # ---- final loss ----
            nst = sm.tile([P, NPP], bf16, tag="nst")
            nc.sync.dma_start(out=nst[:, :], in_=accr_2d)
            nc.vector.tensor_tensor(
                out=scr[:, 0:NPP], in0=nst[:, :], in1=nst[:, :], op=OP.mult)
            nc.vector.tensor_reduce(
                out=red[:, :], in_=scr[:, 0:NPP], axis=mybir.AxisListType.X,
                op=OP.add)
            kclp = ps.tile([1, 1], f32, tag="kclp")
            nc.tensor.matmul(kclp[:, :], lhsT=ones[:, :], rhs=red[:, :],
                             start=True, stop=True)
            prtfb = sm.tile([1, 8], bf16, tag="prtfb")
            nc.sync.dma_start(out=prtfb[:, :], in_=accr_prt)
            prtf = sm.tile([1, 8], f32, tag="prtf")
            nc.vector.tensor_copy(prtf[:, :], prtfb[:, :])
            kcl = sm.tile([1, 1], f32, tag="kcl")
            nc.vector.tensor_scalar_mul(kcl[:, :], kclp[:, :], 1.0 / N_NODES)
            E = float(N_EDGES)
            meanterm = sm.tile([1, 2], f32, tag="meanterm")
            s1 = prtf[:, :].rearrange("o (a b) -> o a b", b=2)[:, 0:2, 0]
            s2 = prtf[:, :].rearrange("o (a b) -> o a b", b=2)[:, 0:2, 1]
            nc.vector.tensor_tensor(out=meanterm[:, :], in0=s1, in1=s1, op=OP.mult)
            nc.vector.tensor_scalar_mul(meanterm[:, :], meanterm[:, :], -1.0 / E)
            nc.vector.tensor_tensor(
                out=meanterm[:, :], in0=meanterm[:, :], in1=s2, op=OP.add)
            kvl = sm.tile([1, 1], f32, tag="kvl")
            nc.vector.tensor_reduce(
                out=kvl[:, :], in_=meanterm[:, :], axis=mybir.AxisListType.X,
                op=OP.add)
            nc.vector.tensor_scalar_mul(kvl[:, :], kvl[:, :], 0.5 / (E - 1.0))

            res = sm.tile([1, 1], f32, tag="res")
            nc.vector.tensor_tensor(
                out=res[:, :], in0=kcl[:, :], in1=kvl[:, :], op=OP.add)
            nc.sync.dma_start(out=out_d[:, :], in_=res[:, :])

    nc.compile()
    return nc


def _build_side(key, other, rank, logits, params, bfdt):
    part = (key // NPP).astype(np.int64)
    opart = (other // NPP).astype(np.int64)
    ordX = np.lexsort((key, part))
    ks = key[ordX]; pt = part[ordX]; op = opart[ordX]; rk = rank[ordX]
    lg = logits[ordX]; pr = params[ordX]
    cnt = np.bincount(pt, minlength=P)
    assert cnt.max() <= CW, cnt.max()
    start = np.zeros(P, np.int64); start[1:] = np.cumsum(cnt)[:-1]
    j = np.arange(len(ks)) - start[pt]
    m = (ks - pt * NPP).astype(np.int64)
    same = np.zeros(len(ks), bool)
    same[1:] = (pt[1:] == pt[:-1]) & (ks[1:] == ks[:-1])
    newrun = ~same
    runend = np.ones(len(ks), bool)
    runend[:-1] = ~same[1:]
    t2cLO = np.full((P, 784), -1, np.int16)
    t2cHI = np.full((P, 784), -1, np.int16)
    jj, pp, mm = j[newrun], pt[newrun], m[newrun]
    lo = jj < HW_
    t2cLO[pp[lo], mm[lo]] = jj[lo].astype(np.int16)
    t2cHI[pp[~lo], mm[~lo]] = (jj[~lo] - HW_).astype(np.int16)
    omff = np.ones((P, CW), np.float32)
    omff[pt[newrun], j[newrun]] = 0.0
    e2n = np.full((P, CW), -1, np.int16)
    e2n[pt[runend], j[runend]] = m[runend].astype(np.int16)
    # split any run crossing the half boundary so LO/HI halves are
    # independent for scan/compact (extra start at HW_, end at HW_-1)
    ends_j = j[runend]
    cross = (jj < HW_) & (ends_j >= HW_)
    cq, cm = pp[cross], mm[cross]
    omff[cq, HW_] = 0.0
    t2cHI[cq, cm] = 0
    e2n[cq, HW_ - 1] = cm.astype(np.int16)
    bpos = op * WB + rk
    s2b = [np.full((P, CW), -1, np.int16) for _ in range(4)]
    tq = bpos // QW
    for t in range(4):
        sel = tq == t
        s2b[t][pt[sel], j[sel]] = (bpos[sel] - t * QW).astype(np.int16)
    b2xLO = np.full((P, BW), -1, np.int16)
    b2xHI = np.full((P, BW), -1, np.int16)
    lo2 = j < HW_
    b2xLO[pt[lo2], bpos[lo2]] = j[lo2].astype(np.int16)
    b2xHI[pt[~lo2], bpos[~lo2]] = (j[~lo2] - HW_).astype(np.int16)
    logits2d = np.full((P, CW), PADLOG, np.float32)
    logits2d[pt, j] = lg
    params2d = np.zeros((P, 2, CW), np.float32)
    params2d[pt, 0, j] = pr[:, 0]
    params2d[pt, 1, j] = pr[:, 1]
    return dict(t2cLO=t2cLO, t2cHI=t2cHI,
                omff=omff.astype(bfdt), e2n=e2n, s2b=s2b,
                b2xLO=b2xLO, b2xHI=b2xHI,
                logits=logits2d.astype(bfdt),
                params=params2d.reshape(P, 2 * CW).astype(bfdt))


def _make_in_maps(node_features, edge_index, edge_logits, edge_params):
    import ml_dtypes
    bfdt = ml_dtypes.bfloat16
    v = np.asarray(node_features[:, 0], dtype=np.float32)
    src = np.asarray(edge_index[0], dtype=np.int64)
    dst = np.asarray(edge_index[1], dtype=np.int64)
    logits = np.asarray(edge_logits, dtype=np.float32)
    params = np.asarray(edge_params, dtype=np.float32)

    vpad = np.zeros((P, 784), np.float32)
    vpad.reshape(-1)[:0] = 0
    vp = np.zeros(P * 784, np.float32)
    vflat = np.zeros(ACC_ROWS, np.float32)
    vflat[:N_NODES] = v
    vpm = vflat.reshape(P, NPP)
    vp784 = np.zeros((P, 784), np.float32)
    vp784[:, :NPP] = vpm

    in_maps = []
    for k in range(NCORES):
        sl = slice(k * EPC, (k + 1) * EPC)
        s_, d_, lg, pr = src[sl], dst[sl], logits[sl], params[sl]
        q = s_ // NPP
        p = d_ // NPP
        key2 = q * P + p
        order2 = np.argsort(key2, kind="stable")
        sk = key2[order2]
        new2 = np.ones(EPC, bool)
        new2[1:] = sk[1:] != sk[:-1]
        starts2 = np.nonzero(new2)[0]
        grp = np.cumsum(new2) - 1
        rank = np.empty(EPC, np.int64)
        rank[order2] = np.arange(EPC) - starts2[grp]
        assert rank.max() < WB, rank.max()
        S = _build_side(s_, d_, rank, lg, pr, bfdt)
        D = _build_side(d_, s_, rank, lg, pr, bfdt)
        im = {"vloc": vp784.astype(bfdt)}
        for X, Bd in (("S", S), ("D", D)):
            im[f"t2cLO{X}"] = Bd["t2cLO"]
            im[f"t2cHI{X}"] = Bd["t2cHI"]
            im[f"omff{X}"] = Bd["omff"]
            im[f"e2n{X}"] = Bd["e2n"]
            for t in range(4):
                im[f"s2b{t}{X}"] = Bd["s2b"][t]
            im[f"b2xLO{X}"] = Bd["b2xLO"]
            im[f"b2xHI{X}"] = Bd["b2xHI"]
            im[f"logits{X}"] = Bd["logits"]
            im[f"params{X}"] = Bd["params"]
        in_maps.append(im)
    return in_maps


def kernel(node_features, edge_index, edge_logits, edge_params):
    from concourse.bass_utils import run_bass_kernel_spmd
    global _last_in_maps

    if "nc" not in _cache:
        _cache["nc"] = _build()
    nc = _cache["nc"]

    in_maps = _make_in_maps(node_features, edge_index, edge_logits, edge_params)
    _last_in_maps = in_maps

    res = run_bass_kernel_spmd(nc, in_maps, core_ids=list(range(NCORES)))
    return np.float32(res.results[0]["out"][0, 0])
